# revision 1
# baseline (speedup 1.0000x reference)
"""Block-sparse position-wise FFN on Trainium2 (Bass/Tile), 8-core data-parallel.

Strategy:
  - Shard tokens (B*S = 36928) evenly across 8 cores: 4616 tokens/core.
    The FFN is pointwise over tokens and both (masked) weight matrices fit
    in SBUF, so data-parallel needs no collectives.
  - Host prep: apply the 8x8 block masks to W1/W2 (weights+masks are layer
    constants) and pre-transpose to the layouts the PE wants. x is fed in
    its natural [T, DIM] layout and transposed on device via PE-transpose.
  - Per core, fused loop over token chunks (<=511 tokens, PSUM-bank sized):
      xT = transpose(x_chunk)                  (PE transpose + DVE copy)
      h  = gelu(W1m @ xT + b1)                 (fp32r matmuls, ACT gelu+bias)
      out_chunk = (hT as stationary).T @ W2mT + b2   (natural-layout output)
    float32r matmul dtype streams at 1 cycle/row for free dim >= 256
    (plain float32 is 4 cycles/row).
"""

import sys
import types

import numpy as np

# concourse's axon trace path imports antenv.axon_hooks, which this image
# lacks; install a no-op shim so an env-requested trace degrades gracefully
# instead of raising ImportError.
try:
    import antenv.axon_hooks  # noqa: F401
except ImportError:
    import antenv

    _hooks = types.ModuleType("antenv.axon_hooks")
    _hooks._hook = None
    _hooks.set_axon_ntff_profile_hook = (
        lambda h: setattr(_hooks, "_hook", h))
    _hooks.get_axon_ntff_profile_hook = lambda: _hooks._hook
    sys.modules["antenv.axon_hooks"] = _hooks
    antenv.axon_hooks = _hooks

import concourse.bass as bass
import concourse.bacc as bacc
import concourse.mybir as mybir
from concourse import tile, masks
from concourse.bass_utils import run_bass_kernel_spmd

B, S, DIM, FF, BLK = 64, 577, 768, 3072, 8
NCORES = 8
TOK = B * S                # 36928
T = TOK // NCORES          # 4616 tokens per core
P = 128
KD = DIM // P              # 6 k-tiles for fc1
KF = FF // P               # 24 f-tiles
F32 = mybir.dt.float32
F32R = mybir.dt.float32r
GELU = mybir.ActivationFunctionType.Gelu


def _chunks(total):
    """Token chunks: 256 wide until the tail (256..511) so every fc1 matmul
    keeps free dim >= 256 (fp32r full rate) and <= 511 (one PSUM bank)."""
    out, pos = [], 0
    while pos < total:
        w = 256 if total - pos >= 512 else total - pos
        out.append((pos, w))
        pos += w
    return out


def _token_tiles(w):
    tiles, off = [], 0
    while off < w:
        p = min(P, w - off)
        tiles.append((off, p))
        off += p
    return tiles


def _body(tc, x_d, w1_d, b1_d, w2_d, b2_d, o_d, t_tokens):
    nc = tc.nc
    with (
        tc.tile_pool(name="const", bufs=1) as constp,
        tc.tile_pool(name="wpool", bufs=1) as wp,
        tc.tile_pool(name="xnat", bufs=4) as xnatp,
        tc.tile_pool(name="xt", bufs=2) as xtp,
        tc.tile_pool(name="ht", bufs=26) as htp,
        tc.tile_pool(name="onat", bufs=2) as onatp,
        tc.tile_pool(name="pst", bufs=1, space=bass.MemorySpace.PSUM) as pstp,
        tc.tile_pool(name="ps1", bufs=3, space=bass.MemorySpace.PSUM) as ps1p,
        tc.tile_pool(name="ps2", bufs=2, space=bass.MemorySpace.PSUM) as ps2p,
    ):
        ident_f = constp.tile([P, P], F32)
        masks.make_identity(nc, ident_f[:])
        ident = constp.tile([P, P], F32R)
        nc.vector.tensor_copy(ident[:], ident_f[:])
        b1_s = constp.tile([P, KF], F32)
        nc.sync.dma_start(out=b1_s[:], in_=b1_d)
        b2_s = constp.tile([P, DIM], F32)
        nc.sync.dma_start(out=b2_s[:], in_=b2_d)

        w1_s = []
        for k in range(KD):
            w = wp.tile([P, FF], F32R, tag=f"w1_{k}", name=f"w1_{k}")
            w1_s.append(w)
        W1CHUNK = FF // 4
        for cc in range(4):
            for k in range(KD):
                nc.sync.dma_start(
                    out=w1_s[k][:, cc * W1CHUNK:(cc + 1) * W1CHUNK],
                    in_=w1_d[k * P:(k + 1) * P,
                             cc * W1CHUNK:(cc + 1) * W1CHUNK],
                )
        w2_s = []
        for k in range(KF):
            w = wp.tile([P, DIM], F32R, tag=f"w2_{k}")
            nc.sync.dma_start(out=w[:], in_=w2_d[k * P:(k + 1) * P, :])
            w2_s.append(w)

        chunks = _chunks(t_tokens)

        def load_transpose(c0, cw):
            """DMA a token chunk and PE-transpose it into xT tiles."""
            xts = [xtp.tile([P, cw], F32R, tag=f"xt{k}", name=f"xt{k}")
                   for k in range(KD)]
            for (toff, tp) in _token_tiles(cw):
                xn = xnatp.tile([P, DIM], F32R, tag="xn", name="xn")
                nc.gpsimd.dma_start(
                    out=xn[0:tp, :], in_=x_d[c0 + toff:c0 + toff + tp, :]
                )
                for k in range(KD):
                    pst = pstp.tile([P, P], F32R, tag="pst", name="pst")
                    nc.tensor.transpose(
                        pst[:, 0:tp], xn[0:tp, k * P:(k + 1) * P],
                        ident[0:tp, 0:tp],
                    )
                    nc.vector.tensor_copy(
                        xts[k][:, toff:toff + tp], pst[:, 0:tp]
                    )
            return xts

        xts = load_transpose(*chunks[0])
        for ci, (c0, cw) in enumerate(chunks):
            ttiles = _token_tiles(cw)

            # --- fc1: hT[m] = gelu(W1mT[:,m].T @ xT + b1[m]) ---
            hts = []
            for m in range(KF):
                ps1 = ps1p.tile([P, cw], F32, tag="ps1")
                for k in range(KD):
                    nc.tensor.matmul(
                        ps1[:, :],
                        w1_s[k][:, m * P:(m + 1) * P],
                        xts[k][:, :],
                        start=(k == 0), stop=(k == KD - 1),
                    )
                ht = htp.tile([P, cw], F32R, tag="ht")
                nc.scalar.activation(
                    ht[:, :], ps1[:, :], GELU, bias=b1_s[:, m:m + 1]
                )
                hts.append(ht)

            # --- prefetch+transpose next chunk while fc2 runs ---
            next_xts = (load_transpose(*chunks[ci + 1])
                        if ci + 1 < len(chunks) else None)

            # --- fc2: out[t, :] = hT.T @ W2mT + b2, natural layout ---
            for (toff, tp) in ttiles:
                ps2 = ps2p.tile([P, DIM], F32, tag="ps2")
                for k in range(KF):
                    last = (k == KF - 1)
                    for off, wdt in ((0, 512), (512, DIM - 512)):
                        nc.tensor.matmul(
                            ps2[0:tp, off:off + wdt],
                            hts[k][:, toff:toff + tp],
                            w2_s[k][:, off:off + wdt],
                            start=(k == 0), stop=last,
                        )
                on = onatp.tile([P, DIM], F32, tag="on")
                nc.vector.tensor_tensor(
                    out=on[0:tp, :], in0=ps2[0:tp, :], in1=b2_s[0:tp, :],
                    op=mybir.AluOpType.add,
                )
                nc.sync.dma_start(
                    out=o_d[c0 + toff:c0 + toff + tp, :], in_=on[0:tp, :]
                )
            xts = next_xts


def build_program(t_tokens=T):
    nc = bacc.Bacc("TRN2", target_bir_lowering=False, debug=False,
                   num_devices=NCORES)
    x_d = nc.dram_tensor("x", [t_tokens, DIM], F32R, kind="ExternalInput").ap()
    w1_d = nc.dram_tensor("w1t", [DIM, FF], F32R, kind="ExternalInput").ap()
    b1_d = nc.dram_tensor("b1", [P, KF], F32, kind="ExternalInput").ap()
    w2_d = nc.dram_tensor("w2t", [FF, DIM], F32R, kind="ExternalInput").ap()
    b2_d = nc.dram_tensor("b2", [P, DIM], F32, kind="ExternalInput").ap()
    o_d = nc.dram_tensor("out", [t_tokens, DIM], F32, kind="ExternalOutput").ap()
    with tile.TileContext(nc) as tc:
        _body(tc, x_d, w1_d, b1_d, w2_d, b2_d, o_d, t_tokens)
    nc.compile()
    return nc


def _round_fp32r(a):
    """Round fp32 values to the fp32r grid (low 12 mantissa bits dropped,
    round-to-nearest), matching the PE's fp32r operand rounding."""
    u = a.view(np.uint32)
    u = (u + np.uint32(0x800)) & np.uint32(0xFFFFF000)
    return u.view(np.float32)


def host_prep(x, W1, b1, W2, b2, mask1, mask2):
    x = _round_fp32r(np.ascontiguousarray(
        np.asarray(x, dtype=np.float32).reshape(TOK, DIM)))
    m1 = np.repeat(np.repeat(np.asarray(mask1, dtype=bool), BLK, 0), BLK, 1)
    m2 = np.repeat(np.repeat(np.asarray(mask2, dtype=bool), BLK, 0), BLK, 1)
    w1t = _round_fp32r(np.ascontiguousarray(
        (np.asarray(W1, np.float32) * m1.astype(np.float32)).T))  # [DIM, FF]
    w2t = _round_fp32r(np.ascontiguousarray(
        (np.asarray(W2, np.float32) * m2.astype(np.float32)).T))  # [FF, DIM]
    b1h = np.ascontiguousarray(
        np.asarray(b1, np.float32).reshape(KF, P).T)              # [P, KF]
    b2h = np.ascontiguousarray(
        np.broadcast_to(np.asarray(b2, np.float32)[None, :], (P, DIM)))
    return x, w1t, b1h, w2t, b2h


_PROGRAM = None


def _get_program():
    global _PROGRAM
    if _PROGRAM is None:
        _PROGRAM = build_program(T)
    return _PROGRAM


def kernel(x, W1, b1, W2, b2, mask1, mask2, **run_kwargs):
    xs, w1t, b1h, w2t, b2h = host_prep(x, W1, b1, W2, b2, mask1, mask2)
    nc = _get_program()
    in_maps = [
        {"x": xs[c * T:(c + 1) * T], "w1t": w1t, "b1": b1h,
         "w2t": w2t, "b2": b2h}
        for c in range(NCORES)
    ]
    res = run_bass_kernel_spmd(nc, in_maps, list(range(NCORES)), **run_kwargs)
    out = np.concatenate([res.results[c]["out"] for c in range(NCORES)], axis=0)
    out = out.reshape(B, S, DIM).astype(np.float32)
    if run_kwargs:
        kernel.last_results = res
    return out



# revision 2
# speedup vs baseline: 1.3681x; 1.3681x over previous
"""Block-sparse position-wise FFN on Trainium2 (Bass/Tile), 8-core data-parallel.

Strategy (v2 — dense bf16 streaming):
  - Shard tokens (B*S = 36928) across 8 cores: 4616 tokens/core. Pointwise
    FFN + weights fit in SBUF => data-parallel, no collectives.
  - The 8x8 block sparsity is applied on host (weights are masked there) but
    NOT exploited on device: random 10%-dense 8x8 blocks aggregate to ~81%
    density at any 32/128-wide PE granularity, so skipping buys nothing.
  - All device data is bf16 (PSUM accumulation stays fp32). bf16 streams at
    1 cycle/row at ANY free size and enables fast-weight-load (2 cols/cyc),
    so the per-matmul LDWEIGHTS (~53ns) hides fully under N=512 matmuls
    (~213ns). fp32r LDW (no FWL) was the baseline's hidden 30% cost.
  - Host pre-transposes x so the device consumes xT [768, T] directly:
    no PE transposes at all. Both layers keep weights stationary:
      fc1: hT[m]  = gelu(w1t[k,:][:, m].T @ xT[k]   + b1), accumulate k=0..5
      fc2: outT[o] =      w2t[k,:][:, o].T @ hT[k] + b2,  accumulate k=0..23
    Output is written transposed [768, T]; host untransposes (free).
"""

import sys
import types

import numpy as np
import ml_dtypes

# concourse's axon trace path imports antenv.axon_hooks, which this image
# lacks; install a no-op shim so an env-requested trace degrades gracefully
# instead of raising ImportError.
try:
    import antenv.axon_hooks  # noqa: F401
except ImportError:
    import antenv

    _hooks = types.ModuleType("antenv.axon_hooks")
    _hooks._hook = None
    _hooks.set_axon_ntff_profile_hook = (
        lambda h: setattr(_hooks, "_hook", h))
    _hooks.get_axon_ntff_profile_hook = lambda: _hooks._hook
    sys.modules["antenv.axon_hooks"] = _hooks
    antenv.axon_hooks = _hooks

import concourse.bass as bass
import concourse.bacc as bacc
import concourse.mybir as mybir
from concourse import tile
from concourse.bass_utils import run_bass_kernel_spmd

B, S, DIM, FF, BLK = 64, 577, 768, 3072, 8
NCORES = 8
TOK = B * S                # 36928
T = TOK // NCORES          # 4616 tokens per core
P = 128
KD = DIM // P              # 6 contraction tiles for fc1 / output tiles fc2
KF = FF // P               # 24 ff tiles
CW = 512                   # chunk width (one PSUM bank of fp32)
F32 = mybir.dt.float32
BF16 = mybir.dt.bfloat16
GELU = mybir.ActivationFunctionType.Gelu

# 7x512 + 3x344 = 4616; all chunks wide enough to amortize dispatch
CHUNKS = [512] * 7 + [344] * 3
assert sum(CHUNKS) == T


def _body(tc, xt_d, w1_d, b1_d, w2_d, b2_d, o_d):
    nc = tc.nc
    with (
        tc.tile_pool(name="const", bufs=1) as constp,
        tc.tile_pool(name="wpool", bufs=1) as wp,
        tc.tile_pool(name="xt", bufs=2) as xtp,
        tc.tile_pool(name="h", bufs=2) as hp,
        tc.tile_pool(name="onat", bufs=2) as onatp,
        tc.tile_pool(name="ps1", bufs=3, space=bass.MemorySpace.PSUM) as ps1p,
        tc.tile_pool(name="ps2", bufs=2, space=bass.MemorySpace.PSUM) as ps2p,
    ):
        b1_s = constp.tile([P, KF], F32)
        nc.sync.dma_start(out=b1_s[:], in_=b1_d)
        b2_s = constp.tile([P, KD], F32)
        nc.sync.dma_start(out=b2_s[:], in_=b2_d)

        w1_s = []
        for k in range(KD):
            w = wp.tile([P, FF], BF16, tag=f"w1_{k}", name=f"w1_{k}")
            nc.sync.dma_start(out=w[:], in_=w1_d[k * P:(k + 1) * P, :])
            w1_s.append(w)
        w2_s = []
        for k in range(KF):
            w = wp.tile([P, DIM], BF16, tag=f"w2_{k}", name=f"w2_{k}")
            nc.sync.dma_start(out=w[:], in_=w2_d[k * P:(k + 1) * P, :])
            w2_s.append(w)

        def load_x(c0, cw):
            xts = [xtp.tile([P, CW], BF16, tag=f"xt{k}", name=f"xt{k}")
                   for k in range(KD)]
            for k in range(KD):
                nc.gpsimd.dma_start(
                    out=xts[k][:, 0:cw],
                    in_=xt_d[k * P:(k + 1) * P, c0:c0 + cw],
                )
            return xts

        starts = [sum(CHUNKS[:i]) for i in range(len(CHUNKS))]
        xts = load_x(starts[0], CHUNKS[0])
        for ci, (c0, cw) in enumerate(zip(starts, CHUNKS)):
            # prefetch next chunk's xT while this chunk computes
            next_xts = (load_x(starts[ci + 1], CHUNKS[ci + 1])
                        if ci + 1 < len(CHUNKS) else None)

            # --- fc1: hT[m] = gelu(W1m slice.T @ xT + b1[m]) ---
            hts = []
            for m in range(KF):
                ps = ps1p.tile([P, CW], F32, tag="ps1", name="ps1")
                for k in range(KD):
                    nc.tensor.matmul(
                        ps[:, 0:cw],
                        w1_s[k][:, m * P:(m + 1) * P],
                        xts[k][:, 0:cw],
                        start=(k == 0), stop=(k == KD - 1),
                    )
                ht = hp.tile([P, CW], BF16, tag=f"h{m}", name=f"h{m}")
                nc.scalar.activation(
                    ht[:, 0:cw], ps[:, 0:cw], GELU, bias=b1_s[:, m:m + 1]
                )
                hts.append(ht)

            # --- fc2: outT[o] = W2m slice.T @ hT + b2[o] ---
            for o in range(KD):
                ps = ps2p.tile([P, CW], F32, tag="ps2", name="ps2")
                for k in range(KF):
                    nc.tensor.matmul(
                        ps[:, 0:cw],
                        w2_s[k][:, o * P:(o + 1) * P],
                        hts[k][:, 0:cw],
                        start=(k == 0), stop=(k == KF - 1),
                    )
                ot = onatp.tile([P, CW], BF16, tag=f"o{o}", name=f"o{o}")
                nc.vector.tensor_scalar_add(
                    ot[:, 0:cw], ps[:, 0:cw], b2_s[:, o:o + 1]
                )
                nc.sync.dma_start(
                    out=o_d[o * P:(o + 1) * P, c0:c0 + cw], in_=ot[:, 0:cw]
                )
            xts = next_xts


def build_program(t_tokens=T):
    nc = bacc.Bacc("TRN2", target_bir_lowering=False, debug=False,
                   num_devices=NCORES)
    xt_d = nc.dram_tensor("xt", [DIM, t_tokens], BF16,
                          kind="ExternalInput").ap()
    w1_d = nc.dram_tensor("w1t", [DIM, FF], BF16, kind="ExternalInput").ap()
    b1_d = nc.dram_tensor("b1", [P, KF], F32, kind="ExternalInput").ap()
    w2_d = nc.dram_tensor("w2t", [FF, DIM], BF16, kind="ExternalInput").ap()
    b2_d = nc.dram_tensor("b2", [P, KD], F32, kind="ExternalInput").ap()
    o_d = nc.dram_tensor("outt", [DIM, t_tokens], BF16,
                         kind="ExternalOutput").ap()
    with tile.TileContext(nc) as tc:
        _body(tc, xt_d, w1_d, b1_d, w2_d, b2_d, o_d)
    nc.compile()
    return nc


def host_prep(x, W1, b1, W2, b2, mask1, mask2):
    m1 = np.repeat(np.repeat(np.asarray(mask1, dtype=bool), BLK, 0), BLK, 1)
    m2 = np.repeat(np.repeat(np.asarray(mask2, dtype=bool), BLK, 0), BLK, 1)
    xt = np.ascontiguousarray(
        np.asarray(x, np.float32).reshape(TOK, DIM).T
    ).astype(ml_dtypes.bfloat16)                                  # [DIM, TOK]
    w1t = np.ascontiguousarray(
        (np.asarray(W1, np.float32) * m1.astype(np.float32)).T
    ).astype(ml_dtypes.bfloat16)                                  # [DIM, FF]
    w2t = np.ascontiguousarray(
        (np.asarray(W2, np.float32) * m2.astype(np.float32)).T
    ).astype(ml_dtypes.bfloat16)                                  # [FF, DIM]
    b1h = np.ascontiguousarray(
        np.asarray(b1, np.float32).reshape(KF, P).T)              # [P, KF]
    b2h = np.ascontiguousarray(
        np.asarray(b2, np.float32).reshape(KD, P).T)              # [P, KD]
    return xt, w1t, b1h, w2t, b2h


_PROGRAM = None


def _get_program():
    global _PROGRAM
    if _PROGRAM is None:
        _PROGRAM = build_program(T)
    return _PROGRAM


def kernel(x, W1, b1, W2, b2, mask1, mask2, **run_kwargs):
    xt, w1t, b1h, w2t, b2h = host_prep(x, W1, b1, W2, b2, mask1, mask2)
    nc = _get_program()
    in_maps = [
        {"xt": np.ascontiguousarray(xt[:, c * T:(c + 1) * T]),
         "w1t": w1t, "b1": b1h, "w2t": w2t, "b2": b2h}
        for c in range(NCORES)
    ]
    res = run_bass_kernel_spmd(nc, in_maps, list(range(NCORES)), **run_kwargs)
    outt = np.concatenate(
        [np.asarray(res.results[c]["outt"]) for c in range(NCORES)], axis=1
    )                                                             # [DIM, TOK]
    out = outt.T.astype(np.float32).reshape(B, S, DIM)
    if run_kwargs:
        kernel.last_results = res
    return out


# revision 3
# speedup vs baseline: 1.4993x; 1.0960x over previous
"""Block-sparse position-wise FFN on Trainium2 (Bass/Tile), 8-core data-parallel.

Strategy (v3 — dense bf16 streaming + permutation-based block skipping):
  - Shard tokens (B*S = 36928) across 8 cores: 4616 tokens/core. Pointwise
    FFN + weights fit in SBUF => data-parallel, no collectives.
  - All device data is bf16 (PSUM accumulation fp32). bf16 streams at
    1 cycle/row at ANY free size and enables fast-weight-load, so the
    per-matmul LDWEIGHTS (~53ns) hides fully under N=512 matmuls (~213ns).
  - Host pre-transposes x; the device consumes xT [768, T] directly (no PE
    transposes). Both layers keep weights stationary:
      fc1: hT[m]   = gelu(w1t[k][:, m].T @ xT[k]  + b1), accumulate over k
      fc2: outT[o] =      w2t[k][:, o].T @ hT[k] + b2,  accumulate over k
    Output is written transposed [768, T]; host untransposes (free).
  - Sparsity: random 10%-dense 8x8 blocks aggregate to ~80% density at any
    128-wide PE tile, so generic skipping is impossible. BUT a host-chosen
    global permutation of ff/dim/out BLOCKS can pack mask-dead rows into
    whole 128x128 stationary tiles: a greedy co-clustering packs f-blocks
    that share a dead contraction k-tile into the same m-tile (and o-blocks
    likewise), making ~20+ of the 288 stationary tiles exactly zero =>
    those matmuls are simply not emitted (~7% less PE work).
"""

import sys
import types

import numpy as np
import ml_dtypes

# concourse's axon trace path imports antenv.axon_hooks, which this image
# lacks; install a no-op shim so an env-requested trace degrades gracefully
# instead of raising ImportError.
try:
    import antenv.axon_hooks  # noqa: F401
except ImportError:
    import antenv

    _hooks = types.ModuleType("antenv.axon_hooks")
    _hooks._hook = None
    _hooks.set_axon_ntff_profile_hook = (
        lambda h: setattr(_hooks, "_hook", h))
    _hooks.get_axon_ntff_profile_hook = lambda: _hooks._hook
    sys.modules["antenv.axon_hooks"] = _hooks
    antenv.axon_hooks = _hooks

import concourse.bass as bass
import concourse.bacc as bacc
import concourse.mybir as mybir
from concourse import tile
from concourse.bass_utils import run_bass_kernel_spmd

B, S, DIM, FF, BLK = 64, 577, 768, 3072, 8
NCORES = 8
TOK = B * S                # 36928
T = TOK // NCORES          # 4616 tokens per core
P = 128
KD = DIM // P              # 6 contraction tiles for fc1 / output tiles fc2
KF = FF // P               # 24 ff tiles
CW = 512                   # chunk width (one PSUM bank of fp32)
F32 = mybir.dt.float32
BF16 = mybir.dt.bfloat16
GELU = mybir.ActivationFunctionType.Gelu

# 7x512 + 3x344 = 4616; all chunks wide enough to amortize dispatch
CHUNKS = [512] * 7 + [344] * 3
assert sum(CHUNKS) == T

NF, ND, NO = FF // BLK, DIM // BLK, DIM // BLK   # 384, 96, 96 blocks
FT, DT, OT = KF, KD, KD                          # 24, 6, 6 tiles
BPT = P // BLK                                   # 16 blocks per tile


# ---------------------------------------------------------------------------
# Host-side permutation search: pack mask-dead blocks into whole zero tiles.
# ---------------------------------------------------------------------------

def _greedy_fgroups(dead1):
    """dead1 [NF, DT] bool -> f-block -> m-tile, packing whole dead tiles."""
    fg = -np.ones(NF, np.int32)
    tilei = 0
    remaining = np.ones(NF, bool)
    # tiles dead for a PAIR of k's first (worth 2 skips each)
    for ka in range(DT):
        for kb in range(ka + 1, DT):
            while tilei < FT:
                cand = np.where(remaining & dead1[:, ka] & dead1[:, kb])[0]
                if len(cand) < BPT:
                    break
                pick = cand[:BPT]
                fg[pick] = tilei
                remaining[pick] = False
                tilei += 1
    # single-k tiles, k by descending availability
    while tilei < FT:
        counts = sorted(((dead1[remaining, k].sum(), k) for k in range(DT)),
                        reverse=True)
        n, k = counts[0]
        if n < BPT:
            break
        cand = np.where(remaining & dead1[:, k])[0]
        other = dead1[cand].sum(1)   # prefer blocks with fewest other dead-k
        pick = cand[np.argsort(other, kind="stable")][:BPT]
        fg[pick] = tilei
        remaining[pick] = False
        tilei += 1
    left = np.where(remaining)[0]
    pos = 0
    for t in range(FT):
        space = BPT - int((fg == t).sum())
        if space > 0:
            fg[left[pos:pos + space]] = t
            pos += space
    return fg


def _greedy_ogroups(dead2):
    """dead2 [NO, FT] bool -> o-block -> o-tile (6 tiles of 16)."""
    og = -np.ones(NO, np.int32)
    remaining = np.ones(NO, bool)
    tilei = 0
    counts = sorted(((dead2[:, ft].sum(), ft) for ft in range(FT)),
                    reverse=True)
    for n, ft in counts:
        if tilei >= OT:
            break
        cand = np.where(remaining & dead2[:, ft])[0]
        if len(cand) < BPT:
            continue
        other = dead2[cand].sum(1)
        pick = cand[np.argsort(other, kind="stable")][:BPT]
        og[pick] = tilei
        remaining[pick] = False
        tilei += 1
    left = np.where(remaining)[0]
    pos = 0
    for t in range(OT):
        space = BPT - int((og == t).sum())
        if space > 0:
            og[left[pos:pos + space]] = t
            pos += space
    return og


def plan_permutation(mask1, mask2):
    m1 = np.asarray(mask1, bool)   # [NF, ND]
    m2 = np.asarray(mask2, bool)   # [NO, NF]
    dg = np.arange(ND) // BPT      # natural d-grouping
    dead1 = np.stack([~m1[:, dg == k].any(1) for k in range(DT)], 1)
    fg = _greedy_fgroups(dead1)
    dead2 = np.stack([~m2[:, fg == t].any(1) for t in range(FT)], 1)
    og = _greedy_ogroups(dead2)

    def perm(g, ntiles):
        return np.concatenate([np.where(g == t)[0] for t in range(ntiles)])

    fperm, dperm, operm = perm(fg, FT), perm(dg, DT), perm(og, OT)
    p1 = m1[np.ix_(fperm, dperm)]
    p2 = m2[np.ix_(operm, fperm)]
    skip1 = frozenset(
        (mt, kt) for mt in range(FT) for kt in range(DT)
        if not p1[mt*BPT:(mt+1)*BPT, kt*BPT:(kt+1)*BPT].any())
    skip2 = frozenset(
        (ot, ft) for ot in range(OT) for ft in range(FT)
        if not p2[ot*BPT:(ot+1)*BPT, ft*BPT:(ft+1)*BPT].any())

    def expand(p):
        return (p[:, None] * BLK + np.arange(BLK)[None, :]).ravel()

    return expand(fperm), expand(dperm), expand(operm), skip1, skip2


# ---------------------------------------------------------------------------
# Device program
# ---------------------------------------------------------------------------

def _body(tc, xt_d, w1_d, b1_d, w2_d, b2_d, o_d, skip1, skip2):
    nc = tc.nc
    with (
        tc.tile_pool(name="const", bufs=1) as constp,
        tc.tile_pool(name="wpool", bufs=1) as wp,
        tc.tile_pool(name="xt", bufs=2) as xtp,
        tc.tile_pool(name="h", bufs=2) as hp,
        tc.tile_pool(name="onat", bufs=2) as onatp,
        tc.tile_pool(name="ps1", bufs=3, space=bass.MemorySpace.PSUM) as ps1p,
        tc.tile_pool(name="ps2", bufs=3, space=bass.MemorySpace.PSUM) as ps2p,
    ):
        b1_s = constp.tile([P, KF], F32)
        nc.sync.dma_start(out=b1_s[:], in_=b1_d)
        b2_s = constp.tile([P, KD], F32)
        nc.sync.dma_start(out=b2_s[:], in_=b2_d)

        # w1 loaded in m-column-major quarters so the first m-tiles'
        # stationaries arrive after ~1.2MB instead of the full 4.7MB
        w1_s = [wp.tile([P, FF], BF16, tag=f"w1_{k}", name=f"w1_{k}")
                for k in range(KD)]
        W1Q = FF // 4
        for q in range(4):
            for k in range(KD):
                nc.sync.dma_start(
                    out=w1_s[k][:, q * W1Q:(q + 1) * W1Q],
                    in_=w1_d[k * P:(k + 1) * P, q * W1Q:(q + 1) * W1Q],
                )
        w2_s = []
        for k in range(KF):
            w = wp.tile([P, DIM], BF16, tag=f"w2_{k}", name=f"w2_{k}")
            nc.sync.dma_start(out=w[:], in_=w2_d[k * P:(k + 1) * P, :])
            w2_s.append(w)

        def load_x(c0, cw):
            xts = [xtp.tile([P, CW], BF16, tag=f"xt{k}", name=f"xt{k}")
                   for k in range(KD)]
            for k in range(KD):
                nc.gpsimd.dma_start(
                    out=xts[k][:, 0:cw],
                    in_=xt_d[k * P:(k + 1) * P, c0:c0 + cw],
                )
            return xts

        fc1_ks = [[k for k in range(KD) if (m, k) not in skip1] or [0]
                  for m in range(KF)]
        fc2_ks = [[k for k in range(KF) if (o, k) not in skip2] or [0]
                  for o in range(KD)]

        starts = [sum(CHUNKS[:i]) for i in range(len(CHUNKS))]
        xts = load_x(starts[0], CHUNKS[0])
        for ci, (c0, cw) in enumerate(zip(starts, CHUNKS)):
            # prefetch next chunk's xT while this chunk computes
            next_xts = (load_x(starts[ci + 1], CHUNKS[ci + 1])
                        if ci + 1 < len(CHUNKS) else None)

            # --- fc1: hT[m] = gelu(W1m slice.T @ xT + b1[m]) ---
            hts = []
            for m in range(KF):
                ps = ps1p.tile([P, CW], F32, tag="ps1", name="ps1")
                ks = fc1_ks[m]
                for k in ks:
                    nc.tensor.matmul(
                        ps[:, 0:cw],
                        w1_s[k][:, m * P:(m + 1) * P],
                        xts[k][:, 0:cw],
                        start=(k == ks[0]), stop=(k == ks[-1]),
                    )
                ht = hp.tile([P, CW], BF16, tag=f"h{m}", name=f"h{m}")
                nc.scalar.activation(
                    ht[:, 0:cw], ps[:, 0:cw], GELU, bias=b1_s[:, m:m + 1]
                )
                hts.append(ht)

            # --- fc2: outT[o] = W2m slice.T @ hT + b2[o] ---
            for o in range(KD):
                ps = ps2p.tile([P, CW], F32, tag="ps2", name="ps2")
                ks = fc2_ks[o]
                for k in ks:
                    nc.tensor.matmul(
                        ps[:, 0:cw],
                        w2_s[k][:, o * P:(o + 1) * P],
                        hts[k][:, 0:cw],
                        start=(k == ks[0]), stop=(k == ks[-1]),
                    )
                ot = onatp.tile([P, CW], BF16, tag=f"o{o}", name=f"o{o}")
                nc.vector.tensor_scalar_add(
                    ot[:, 0:cw], ps[:, 0:cw], b2_s[:, o:o + 1]
                )
                nc.sync.dma_start(
                    out=o_d[o * P:(o + 1) * P, c0:c0 + cw], in_=ot[:, 0:cw]
                )
            xts = next_xts


def build_program(skip1, skip2, t_tokens=T):
    nc = bacc.Bacc("TRN2", target_bir_lowering=False, debug=False,
                   num_devices=NCORES)
    xt_d = nc.dram_tensor("xt", [DIM, t_tokens], BF16,
                          kind="ExternalInput").ap()
    w1_d = nc.dram_tensor("w1t", [DIM, FF], BF16, kind="ExternalInput").ap()
    b1_d = nc.dram_tensor("b1", [P, KF], F32, kind="ExternalInput").ap()
    w2_d = nc.dram_tensor("w2t", [FF, DIM], BF16, kind="ExternalInput").ap()
    b2_d = nc.dram_tensor("b2", [P, KD], F32, kind="ExternalInput").ap()
    o_d = nc.dram_tensor("outt", [DIM, t_tokens], BF16,
                         kind="ExternalOutput").ap()
    with tile.TileContext(nc) as tc:
        _body(tc, xt_d, w1_d, b1_d, w2_d, b2_d, o_d, skip1, skip2)
    nc.compile()
    return nc


def host_prep(x, W1, b1, W2, b2, mask1, mask2, fpe, dpe, ope):
    m1 = np.repeat(np.repeat(np.asarray(mask1, dtype=bool), BLK, 0), BLK, 1)
    m2 = np.repeat(np.repeat(np.asarray(mask2, dtype=bool), BLK, 0), BLK, 1)
    xt = np.ascontiguousarray(
        np.asarray(x, np.float32).reshape(TOK, DIM).T[dpe]
    ).astype(ml_dtypes.bfloat16)                                  # [DIM, TOK]
    wm1t = (np.asarray(W1, np.float32) * m1.astype(np.float32)).T
    w1t = np.ascontiguousarray(
        wm1t[np.ix_(dpe, fpe)]).astype(ml_dtypes.bfloat16)        # [DIM, FF]
    wm2t = (np.asarray(W2, np.float32) * m2.astype(np.float32)).T
    w2t = np.ascontiguousarray(
        wm2t[np.ix_(fpe, ope)]).astype(ml_dtypes.bfloat16)        # [FF, DIM]
    b1h = np.ascontiguousarray(
        np.asarray(b1, np.float32)[fpe].reshape(KF, P).T)         # [P, KF]
    b2h = np.ascontiguousarray(
        np.asarray(b2, np.float32)[ope].reshape(KD, P).T)         # [P, KD]
    return xt, w1t, b1h, w2t, b2h


_PROGRAM = None
_PROGRAM_KEY = None


def _get_program(skip1, skip2, key):
    global _PROGRAM, _PROGRAM_KEY
    if _PROGRAM is None or _PROGRAM_KEY != key:
        _PROGRAM = build_program(skip1, skip2)
        _PROGRAM_KEY = key
    return _PROGRAM


def kernel(x, W1, b1, W2, b2, mask1, mask2, **run_kwargs):
    fpe, dpe, ope, skip1, skip2 = plan_permutation(mask1, mask2)
    xt, w1t, b1h, w2t, b2h = host_prep(
        x, W1, b1, W2, b2, mask1, mask2, fpe, dpe, ope)
    key = (np.asarray(mask1).tobytes(), np.asarray(mask2).tobytes())
    nc = _get_program(skip1, skip2, key)
    in_maps = [
        {"xt": np.ascontiguousarray(xt[:, c * T:(c + 1) * T]),
         "w1t": w1t, "b1": b1h, "w2t": w2t, "b2": b2h}
        for c in range(NCORES)
    ]
    res = run_bass_kernel_spmd(nc, in_maps, list(range(NCORES)), **run_kwargs)
    outt = np.concatenate(
        [np.asarray(res.results[c]["outt"]) for c in range(NCORES)], axis=1
    )                                                             # [DIM, TOK]
    out = np.empty((TOK, DIM), np.float32)
    out[:, ope] = outt.T.astype(np.float32)
    out = out.reshape(B, S, DIM)
    if run_kwargs:
        kernel.last_results = res
    return out
